# revision 30
# baseline (speedup 1.0000x reference)
"""Self-contained Trainium2 Bass kernel for a 2-layer GAT (GATConv x2, PyG-style).

Contract: kernel(**inputs) takes the FULL inputs (x [N,128] f32, edge_index
[2,E] int, W1/att_src1/att_dst1/b1/W2/att_src2/att_dst2/b2) and returns the
FULL [N,128] f32 output, distributing work across 8 NeuronCores internally.

Strategy (graph-parallel, destination-sharded):
  - Destinations are sharded across the 8 cores; each core owns 6272 padded
    node positions arranged into 49 blocks of 128 (dst = SBUF partition).
  - Per layer every core redundantly computes h_ext = x @ W_ext for ALL nodes
    (x replicated by the host => no exchange before layer 1) and writes 512B
    bf16 rows [h bf16(128) | pad | a_src f32 | pad] to local DRAM.
  - Per block, source features are fetched with dma_gather (int16 indices,
    two 25088-row windows) into [128 dst, t slots, 256] tiles. Slot padding
    points at a dedicated pad row whose a_src is a large negative value, so
    exp() masks padding with no on-chip mask tensor.
  - Softmax runs per-partition on ACT/DVE; the weighted message sum runs on
    the TensorEngine as a chain of diag(alpha) matmuls accumulating in PSUM,
    producing the output TRANSPOSED ([feature, dst]) which feeds layer 2's
    lhsT directly and is transposed back on the host at the end.
  - Between layers, one AllGather (2 chunks) exchanges x2^T bf16 shards.
"""

import hashlib
from contextlib import ExitStack

import ml_dtypes
import numpy as np

# ---------------------------------------------------------------------------
# Configuration
# ---------------------------------------------------------------------------

F = 128              # feature dim (all of F_in, H, F_out)
CORES = 8
ROW = 256            # bf16 elements per h_ext row (512 B)
ACOL = 65            # f32 column (within bitcast row) holding a_src
NEG_SLOPE = 0.2
PAD_S = 2000.0       # pad rows get a_src=-PAD_S, pad dsts a_dst=+PAD_S
GH = 3               # node tiles per h-phase group (PSUM bank holds 3x130 f32)
NQUEUES = 4
GCHUNK = 8           # gathered slot-columns (128 idxs each) per dma_gather


class Cfg:
    def __init__(self, n_nodes, per_core_blocks):
        self.N = n_nodes
        self.NB = per_core_blocks              # blocks of 128 dsts per core
        self.PERP = per_core_blocks * 128      # padded positions per core
        self.PER = n_nodes // CORES            # real nodes per core
        assert self.PER * CORES == n_nodes
        assert self.PERP >= self.PER
        self.NPAD = self.PERP * CORES
        self.WIN = self.PERP * (CORES // 2)    # gather window boundary
        assert self.WIN - 1 < 32768, "window must fit int16"
        self.NT = self.NPAD // 128             # global node tiles
        self.HB = per_core_blocks // 2         # blocks in AllGather chunk A


FULL_CFG = Cfg(50000, 49)


# ---------------------------------------------------------------------------
# Host-side topology preprocessing (pure graph structure, no feature math)
# ---------------------------------------------------------------------------

def build_topology(cfg, edge_index):
    """Returns the shared block structure + per-core gather metadata."""
    src = np.asarray(edge_index[0], dtype=np.int64)
    dst = np.asarray(edge_index[1], dtype=np.int64)
    N, PER, PERP, NB = cfg.N, cfg.PER, cfg.PERP, cfg.NB
    half = N // 2  # orig-id window boundary (cores 0-3 vs 4-7)

    # pad slots gather this position: a pad node (a_src=-PAD_S) in each window
    PAD_A = PER                       # core 0's first pad position, window A
    PAD_B = PERP * (CORES // 2) + PER - cfg.WIN   # core 4's, window B

    # per-core dst permutations and per-block structure
    per_core = []
    orig2pos = np.full(N, -1, dtype=np.int64)
    for k in range(CORES):
        lo = PER * k
        m = (dst >= lo) & (dst < lo + PER)
        s_k = src[m]
        d_k = dst[m] - lo
        w0 = np.bincount(d_k[s_k < half], minlength=PER)
        w1 = np.bincount(d_k[s_k >= half], minlength=PER)
        order = np.lexsort((w0, np.maximum(w0, w1)))
        w0o = np.concatenate([w0[order], np.zeros(PERP - PER, np.int64)])
        w1o = np.concatenate([w1[order], np.zeros(PERP - PER, np.int64)])
        orig2pos[order + lo] = np.arange(PER) + PERP * k
        per_core.append({
            "edges_src": s_k, "edges_dst_local": d_k,
            "S0k": w0o.reshape(NB, 128).max(1),
            "S1k": w1o.reshape(NB, 128).max(1),
        })

    S0 = np.max([c["S0k"] for c in per_core], axis=0).astype(np.int64)
    S1 = np.max([c["S1k"] for c in per_core], axis=0).astype(np.int64)
    T = 1 + S0 + S1                           # self col + both windows

    pos2orig = np.full(cfg.NPAD, -1, dtype=np.int64)
    valid = orig2pos >= 0
    pos2orig[orig2pos[valid]] = np.nonzero(valid)[0]

    # per-core idx arrays
    IA = int(8 * S0.sum())
    IB = int(8 * S1.sum())
    idxA = np.zeros((CORES, 128, max(IA, 16)), np.int16)
    idxB = np.zeros((CORES, 128, max(IB, 16)), np.int16)

    for k in range(CORES):
        c = per_core[k]
        # bucket edges by local dst position
        pos_of_dst = orig2pos[c["edges_dst_local"] + PER * k] - PERP * k
        spos = orig2pos[c["edges_src"]]
        isw0 = spos < cfg.WIN
        bucket0 = [[] for _ in range(PERP)]
        bucket1 = [[] for _ in range(PERP)]
        for p, sp, w in zip(pos_of_dst, spos, isw0):
            (bucket0 if w else bucket1)[p].append(sp)
        aoff = boff = 0
        for b in range(NB):
            s0, s1 = int(S0[b]), int(S1[b])
            flatA = np.full(128 * s0, PAD_A, np.int64)
            flatB = np.full(128 * s1, PAD_B, np.int64)
            for p in range(128):
                g = 128 * b + p
                for s_i, sp in enumerate(bucket0[g]):
                    flatA[s_i * 128 + p] = sp
                for s_i, sp in enumerate(bucket1[g]):
                    flatB[s_i * 128 + p] = sp - cfg.WIN
            # wrap indices into [128, n/16] int16 (16-row layout, replicated)
            for nfl, arr, tgt, off in ((s0, flatA, idxA, aoff),
                                       (s1, flatB, idxB, boff)):
                if nfl == 0:
                    continue
                cols = 8 * nfl
                wrapped = arr.reshape(cols, 16).T.astype(np.int16)
                tgt[k, :, off:off + cols] = np.tile(wrapped, (8, 1))
            aoff += 8 * s0
            boff += 8 * s1

    stats = {
        "real_edges": int(sum(len(c["edges_src"]) for c in per_core)) + N,
        "padded_edges": int((T.sum()) * 128 * CORES),
    }
    return {
        "S0": S0, "S1": S1, "T": T, "IA": IA, "IB": IB,
        "idxA": idxA, "idxB": idxB,
        "orig2pos": orig2pos, "pos2orig": pos2orig, "stats": stats,
    }


# ---------------------------------------------------------------------------
# Bass program
# ---------------------------------------------------------------------------

def build_program(cfg, topo, queue_map=None):
    """queue_map: construction-index -> queue_num. Tile assigns the SWDGE
    DMASW sem lanes in SCHEDULED order; a sem lane is locked to the queue of
    the first gather that bumps it, so queue_num must follow the scheduled
    rotation. We discover it with a first compile pass (see _get_program)."""
    import concourse.bacc as bacc
    import concourse.mybir as mybir
    import concourse.tile as tile

    dt = mybir.dt
    S0, S1, T = topo["S0"], topo["S1"], topo["T"]
    IA, IB = topo["IA"], topo["IB"]
    NPAD, PERP, WIN, NB, NT = cfg.NPAD, cfg.PERP, cfg.WIN, cfg.NB, cfg.NT
    PER = cfg.PER
    # h-row exchange chunks (in blocks): each chunk's AllGather fires as
    # soon as its h_own rows are written; the last chunk is small so its
    # wire time hides behind the earlier chunks' repack + gathers
    c0 = max(1, (4 * NB) // 9)
    CH = [(0, c0), (c0, 2 * c0), (2 * c0, NB)]
    CH = [(a, b) for a, b in CH if b > a]

    nc = bacc.Bacc("TRN2", target_bir_lowering=False, debug=False,
                   enable_asserts=False, num_devices=CORES,
                   num_swdge_queues=NQUEUES,
                   dynamic_dma_scratch_size=32768)

    # --- kernel I/O ---
    xTg = nc.dram_tensor("xTg", [F, NPAD], dt.bfloat16, kind="ExternalInput")
    xTo = nc.dram_tensor("xTo", [F, PERP], dt.bfloat16, kind="ExternalInput")
    W1e = nc.dram_tensor("W1e", [F, 130], dt.bfloat16, kind="ExternalInput")
    W2e = nc.dram_tensor("W2e", [F, 130], dt.bfloat16, kind="ExternalInput")
    idxA_d = nc.dram_tensor("idxA", [128, max(IA, 16)], dt.int16, kind="ExternalInput")
    idxB_d = nc.dram_tensor("idxB", [128, max(IB, 16)], dt.int16, kind="ExternalInput")
    b1c_d = nc.dram_tensor("b1c", [128, 1], dt.float32, kind="ExternalInput")
    b2c_d = nc.dram_tensor("b2c", [128, 1], dt.float32, kind="ExternalInput")
    eyeb_d = nc.dram_tensor("eyeb", [128, 128], dt.bfloat16, kind="ExternalInput")
    x2pad_d = nc.dram_tensor("x2pad", [128, 1], dt.bfloat16, kind="ExternalInput")
    outT_d = nc.dram_tensor("outT", [F, PERP], dt.float32, kind="ExternalOutput")

    # --- internal DRAM ---
    h1x = nc.dram_tensor("h1x", [NPAD, ROW], dt.bfloat16)
    h1o = nc.dram_tensor("h1o", [PERP, ROW], dt.bfloat16)
    h2x = nc.dram_tensor("h2x", [NPAD, ROW], dt.bfloat16)
    h2o = nc.dram_tensor("h2o", [PERP, ROW], dt.bfloat16)
    # packed (132-col) h2 rows: shard staging + AllGather outputs
    cc2_in = nc.dram_tensor("cc2_in", [PERP, 132], dt.bfloat16)
    cc2_outs = [nc.dram_tensor(f"cc2_{i}out",
                               [CORES, 128 * (b - a) * 132], dt.bfloat16,
                               addr_space="Shared")
                for i, (a, b) in enumerate(CH)]

    with tile.TileContext(nc) as tc, ExitStack() as ctx:
        P = ctx.enter_context(tc.tile_pool(name="persist", bufs=1))
        hp = ctx.enter_context(tc.tile_pool(name="hp", bufs=4))
        php = ctx.enter_context(tc.tile_pool(name="php", bufs=2, space="PSUM"))
        php = ctx.enter_context(tc.tile_pool(name="php", bufs=2, space="PSUM"))
        pop = ctx.enter_context(tc.tile_pool(name="pop", bufs=2, space="PSUM"))
        pep = ctx.enter_context(tc.tile_pool(name="pep", bufs=3, space="PSUM"))
        gp = ctx.enter_context(tc.tile_pool(name="gp", bufs=4))
        dp = ctx.enter_context(tc.tile_pool(name="dp", bufs=3))
        sp = ctx.enter_context(tc.tile_pool(name="sp", bufs=6))
        op = ctx.enter_context(tc.tile_pool(name="op", bufs=3))

        # persistent SBUF
        idxA_s = P.tile([128, max(IA, 16)], dt.int16)
        idxB_s = P.tile([128, max(IB, 16)], dt.int16)
        W1e_s = P.tile([F, 130], dt.bfloat16)
        W2e_s = P.tile([F, 130], dt.bfloat16)
        b1c_s = P.tile([128, 1], dt.float32)
        b2c_s = P.tile([128, 1], dt.float32)
        eyeb_s = P.tile([128, 128], dt.bfloat16)
        x2pad_s = P.tile([128, 1], dt.bfloat16)
        adst1 = P.tile([128, NB], dt.float32)
        adst2 = P.tile([128, NB], dt.float32)
        x2t = [P.tile([F, 128 * (b - a)], dt.bfloat16, name=f"x2c{i}")
               for i, (a, b) in enumerate(CH)]

        def x2ap(b):
            for (a, bb), tile_ in zip(CH, x2t):
                if a <= b < bb:
                    return tile_[:, 128 * (b - a):128 * (b - a + 1)]
            raise AssertionError(b)

        nc.sync.dma_start(idxA_s[:], idxA_d[:])
        nc.sync.dma_start(idxB_s[:], idxB_d[:])
        nc.sync.dma_start(W1e_s[:], W1e[:])
        nc.sync.dma_start(W2e_s[:], W2e[:])
        nc.sync.dma_start(b1c_s[:], b1c_d[:])
        nc.sync.dma_start(b2c_s[:], b2c_d[:])
        nc.sync.dma_start(eyeb_s[:], eyeb_d[:])
        nc.sync.dma_start(x2pad_s[:], x2pad_d[:])

        def h_group(xt_ap, hx_dram, row0, gn, We_s, use_act):
            """h_ext for gn node tiles (lhsT columns of xt_ap) -> 512B rows."""
            ps = php.tile([128, GH, 130], dt.float32, tag="ps")
            for j in range(gn):
                nc.tensor.matmul(ps[:, j, :],
                                 xt_ap[:, 128 * j:128 * (j + 1)], We_s[:])
            hxt = hp.tile([128, GH, ROW], dt.bfloat16, tag="hx")
            if use_act:
                nc.scalar.activation(hxt[:, 0:gn, 0:130], ps[:, 0:gn, 0:130],
                                     mybir.ActivationFunctionType.Copy)
            else:
                nc.vector.tensor_copy(hxt[:, 0:gn, 0:130], ps[:, 0:gn, 0:130])
            h32 = hxt[:].bitcast(dt.float32)    # [128, GH, 128]
            nc.vector.tensor_copy(h32[:, 0:gn, ACOL:ACOL + 1],
                                  ps[:, 0:gn, 128:129])
            nc.scalar.dma_start(
                hx_dram[row0:row0 + 128 * gn, 0:132].rearrange(
                    "(g p) c -> p g c", p=128), hxt[:, 0:gn, 0:132])

        def h_global_l1():
            for i, t0 in enumerate(range(0, NT, GH)):
                gn = min(GH, NT - t0)
                xt = hp.tile([F, 128 * GH], dt.bfloat16, tag="xt")
                nc.sync.dma_start(xt[:, 0:128 * gn],
                                  xTg[:, 128 * t0:128 * (t0 + gn)])
                h_group(xt[:], h1x, 128 * t0, gn, W1e_s, use_act=(i % 2 == 0))

        def h_own_block(xsrc_kind, b, ho_dram, cc_in, We_s, adst_s):
            if xsrc_kind == "xTo":
                xt = hp.tile([F, 128], dt.bfloat16, tag="xto")
                nc.sync.dma_start(xt[:], xTo[:, 128 * b:128 * (b + 1)])
                lhs = xt[:]
            else:
                lhs = x2ap(b)
            ps = pop.tile([128, 130], dt.float32, tag="pso")
            nc.tensor.matmul(ps[:], lhs, We_s[:])
            hxt = hp.tile([128, ROW], dt.bfloat16, tag="hxo")
            nc.scalar.activation(hxt[:, 0:130], ps[:, 0:130],
                                 mybir.ActivationFunctionType.Copy)
            h32 = hxt[:].bitcast(dt.float32)
            nc.vector.tensor_copy(h32[:, ACOL:ACOL + 1], ps[:, 128:129])
            nc.vector.tensor_copy(adst_s[:, b:b + 1], ps[:, 129:130])
            nc.scalar.dma_start(ho_dram[128 * b:128 * (b + 1), 0:132],
                                hxt[:, 0:132])
            if cc_in is not None:
                nc.scalar.dma_start(cc_in[128 * b:128 * (b + 1), :],
                                    hxt[:, 0:132])

        def exchange_chunk(i, cc_in, cc_outs_l, hx_dram):
            """AllGather one packed h-row chunk, then DMA-repack every
            rank's rows into the 512B-stride gather layout (window-A ranks
            first)."""
            (b0, b1), cc_out_t = CH[i], cc_outs_l[i]
            rows = 128 * (b1 - b0)
            nc.gpsimd.collective_compute(
                "AllGather", mybir.AluOpType.bypass,
                replica_groups=[list(range(CORES))],
                ins=[cc_in[128 * b0:128 * b1, :].opt()],
                outs=[cc_out_t[:].opt()])
            for r in list(range(CORES // 2)) + list(range(CORES // 2, CORES)):
                nc.sync.dma_start(
                    hx_dram[r * PERP + 128 * b0:
                            r * PERP + 128 * b0 + rows, 0:132],
                    cc_out_t[r, :].rearrange("(n c) -> n c", c=132))

        def exchange(cc_in, cc_outs_l, hx_dram):
            for i in range(len(CH)):
                exchange_chunk(i, cc_in, cc_outs_l, hx_dram)

        gq = [0]
        gather_insts = []

        def qnum():
            i = gq[0]
            gq[0] += 1
            return queue_map.get(i, 0) if queue_map else 0

        def edge_phase(hx_dram, ho_dram, adst_s, bc_s, layer):
            aoff = boff = 0
            for b in range(NB):
                s0, s1, t = int(S0[b]), int(S1[b]), int(T[b])
                G = gp.tile([128, t, ROW], dt.bfloat16, tag="G")
                nc.sync.dma_start(G[:, 0, 0:132],
                                  ho_dram[128 * b:128 * (b + 1), 0:132])
                for c0 in range(0, s0, GCHUNK):
                    cn = min(GCHUNK, s0 - c0)
                    gather_insts.append(nc.gpsimd.dma_gather(
                        G[:, 1 + c0:1 + c0 + cn, :], hx_dram[0:WIN, :],
                        idxA_s[:, aoff + 8 * c0:aoff + 8 * (c0 + cn)],
                        128 * cn, 128 * cn, ROW, queue_num=qnum()))
                for c0 in range(0, s1, GCHUNK):
                    cn = min(GCHUNK, s1 - c0)
                    gather_insts.append(nc.gpsimd.dma_gather(
                        G[:, 1 + s0 + c0:1 + s0 + c0 + cn, :],
                        hx_dram[WIN:NPAD, :],
                        idxB_s[:, boff + 8 * c0:boff + 8 * (c0 + cn)],
                        128 * cn, 128 * cn, ROW, queue_num=qnum()))
                G32 = G[:].bitcast(dt.float32)      # [128, t, 128]
                # e = a_src + a_dst ; leaky ; exp (padding: a_src=-PAD_S -> 0)
                # NB: DVE's AP-scalar operand path costs ~4.3us flat, so feed
                # a_dst as a stride-0 broadcast tensor operand instead.
                E = sp.tile([128, t], dt.float32, tag="E")
                nc.vector.tensor_tensor(
                    E[:], G32[:, :, ACOL],
                    adst_s[:, b:b + 1].broadcast_to([128, t]),
                    mybir.AluOpType.add)
                EL = sp.tile([128, t], dt.float32, tag="EL")
                nc.vector.scalar_tensor_tensor(
                    EL[:], E[:], NEG_SLOPE, E[:],
                    mybir.AluOpType.mult, mybir.AluOpType.max)
                EX = sp.tile([128, t], dt.float32, tag="EX")
                den = sp.tile([128, 1], dt.float32, tag="den")
                nc.scalar.activation(EX[:], EL[:],
                                     mybir.ActivationFunctionType.Exp,
                                     accum_out=den[:])
                rec = sp.tile([128, 1], dt.float32, tag="rec")
                nc.vector.reciprocal(rec[:], den[:])
                EXnb = sp.tile([128, t], dt.bfloat16, tag="EXnb")
                nc.scalar.activation(EXnb[:], EX[:],
                                     mybir.ActivationFunctionType.Copy,
                                     scale=rec[:, 0:1])
                # D[:, s, :] = diag-stack of normalized alphas
                D = dp.tile([128, t, F], dt.bfloat16, tag="D")
                nc.vector.tensor_tensor(
                    D[:], EXnb[:].unsqueeze(2).broadcast_to([128, t, F]),
                    eyeb_s[:].unsqueeze(1).broadcast_to([128, t, F]),
                    mybir.AluOpType.mult)
                # psum[f, d] = sum_s G_s^T @ D_s  (transposed GAT output)
                ps = pep.tile([128, 128], dt.float32, tag="ps")
                for s in range(t):
                    nc.tensor.matmul(ps[:], G[:, s, 0:F], D[:, s, :],
                                     start=(s == 0), stop=(s == t - 1))
                if layer == 1:
                    nc.scalar.activation(x2ap(b), ps[:],
                                         mybir.ActivationFunctionType.Relu,
                                         bias=bc_s[:, 0:1])
                    if b < NB - 1:
                        h_own_block("x2", b, h2o, cc2_in, W2e_s, adst2)
                    for i in range(len(CH) - 1):
                        if b == min(CH[i][1] + 2, NB - 2):
                            exchange_chunk(i, cc2_in, cc2_outs, h2x)
                else:
                    ot = op.tile([128, 128], dt.float32, tag="ot")
                    nc.scalar.activation(ot[:], ps[:],
                                         mybir.ActivationFunctionType.Relu,
                                         bias=bc_s[:, 0:1])
                    nc.sync.dma_start(outT_d[:, 128 * b:128 * (b + 1)], ot[:])
                aoff += 8 * s0
                boff += 8 * s1

        # ---- layer 1: replicated local h (engines are idle at start) ----
        h_global_l1()
        for b in range(NB):
            h_own_block("xTo", b, h1o, None, W1e_s, adst1)
        edge_phase(h1x, h1o, adst1, b1c_s, layer=1)
        # (layer-2 h_own for blocks 0..NB-2 is interleaved into edge_phase)

        # pad columns of x2 get the crafted pad vector (a_src2=-S, a_dst2=+S);
        # then the last block's h_own runs over the patched columns
        lastw = 128 * (CH[-1][1] - CH[-1][0])
        nc.vector.tensor_copy(
            x2t[-1][:, lastw - (PERP - PER):lastw],
            x2pad_s[:].broadcast_to([128, PERP - PER]))
        h_own_block("x2", NB - 1, h2o, cc2_in, W2e_s, adst2)

        # ---- layer 2 (chunks 0..n-2 already exchanged mid-edge-phase) ----
        exchange_chunk(len(CH) - 1, cc2_in, cc2_outs, h2x)
        edge_phase(h2x, h2o, adst2, b2c_s, layer=2)

    nc.compile()
    # final (scheduled) order of the gather instructions, by construction idx
    name2ci = {gi.ins.name: ci for ci, gi in enumerate(gather_insts)}
    sched = []

    def _walk(bb):
        for inst in bb.instructions:
            ci = name2ci.get(inst.name)
            if ci is not None:
                sched.append(ci)
            body = getattr(inst, "body_bb", None)
            if body is not None:
                _walk(body)

    for bb in nc.m.functions[0].blocks:
        _walk(bb)
    sched_q = {ci: pos % NQUEUES for pos, ci in enumerate(sched)}
    return nc, sched_q


# ---------------------------------------------------------------------------
# Host orchestration
# ---------------------------------------------------------------------------

def make_inputs(cfg, topo, x, W1, as1, ad1, b1, W2, as2, ad2, b2):
    N, NPAD, PERP, PER = cfg.N, cfg.NPAD, cfg.PERP, cfg.PER
    bf16 = ml_dtypes.bfloat16
    pos2orig = topo["pos2orig"]

    def wext(W, a_s, a_d):
        W = np.asarray(W, np.float64)
        return np.concatenate(
            [W, (W @ np.asarray(a_s, np.float64))[:, None],
             (W @ np.asarray(a_d, np.float64))[:, None]], axis=1)

    W1f = wext(W1, as1, ad1)            # [F, 130] f64
    W2f = wext(W2, as2, ad2)

    def pad_vec(Wf):
        # least-norm x with x@Wa_s = -PAD_S and x@Wa_d = +PAD_S
        A = Wf[:, 128:130].T            # [2, F]
        rhs = np.array([-PAD_S, PAD_S], np.float64)
        return (A.T @ np.linalg.solve(A @ A.T, rhs))

    x1p = pad_vec(W1f)
    x2p = pad_vec(W2f)

    xT = np.empty((F, NPAD), bf16)
    xT[:] = x1p[:, None].astype(bf16)   # pad columns get the mask vector
    valid = pos2orig >= 0
    xT[:, valid] = np.asarray(x, np.float32)[pos2orig[valid]].T.astype(bf16)

    in_maps = []
    for k in range(CORES):
        in_maps.append({
            "xTg": xT,
            "xTo": np.ascontiguousarray(xT[:, PERP * k:PERP * (k + 1)]),
            "W1e": W1f.astype(bf16), "W2e": W2f.astype(bf16),
            "idxA": topo["idxA"][k],
            "idxB": topo["idxB"][k],
            "b1c": np.asarray(b1, np.float32)[:, None],
            "b2c": np.asarray(b2, np.float32)[:, None],
            "eyeb": np.eye(128, dtype=bf16),
            "x2pad": x2p[:, None].astype(bf16),
        })
    return in_maps


_CACHE = {}


def _get_program(cfg, edge_index):
    key = hashlib.sha1(np.ascontiguousarray(edge_index).tobytes()).hexdigest()
    if key not in _CACHE:
        topo = build_topology(cfg, edge_index)
        # pass 1 discovers the scheduler's gather order (=> DMASW sem lanes);
        # pass 2 rebuilds with queue_num matching it. Iterate to convergence
        # in case queue choice perturbs the schedule.
        qmap = None
        for _ in range(4):
            nc, sched_q = build_program(cfg, topo, qmap)
            if qmap == sched_q:
                break
            qmap = sched_q
        _CACHE[key] = (topo, nc)
    return _CACHE[key]


def run(cfg, inputs, trace=False):
    from concourse.bass_utils import run_bass_kernel_spmd

    topo, nc = _get_program(cfg, inputs["edge_index"])
    in_maps = make_inputs(
        cfg, topo, inputs["x"],
        inputs["W1"], inputs["att_src1"], inputs["att_dst1"], inputs["b1"],
        inputs["W2"], inputs["att_src2"], inputs["att_dst2"], inputs["b2"])
    res = run_bass_kernel_spmd(nc, in_maps, list(range(CORES)), trace=trace)

    full = np.zeros((cfg.N, F), np.float32)
    pos2orig = topo["pos2orig"]
    for k in range(CORES):
        o = np.asarray(res.results[k]["outT"], np.float32)   # [F, PERP]
        po = pos2orig[cfg.PERP * k:cfg.PERP * (k + 1)]
        m = po >= 0
        full[po[m]] = o[:, m].T
    return full, res


def kernel(**inputs) -> np.ndarray:
    out, _ = run(FULL_CFG, inputs)
    return out


# revision 31
# speedup vs baseline: 1.1174x; 1.1174x over previous
"""Self-contained Trainium2 Bass kernel for a 2-layer GAT (GATConv x2, PyG-style).

Contract: kernel(**inputs) takes the FULL inputs (x [N,128] f32, edge_index
[2,E] int, W1/att_src1/att_dst1/b1/W2/att_src2/att_dst2/b2) and returns the
FULL [N,128] f32 output, distributing work across 8 NeuronCores internally.

Strategy (graph-parallel, destination-sharded):
  - Destinations are sharded across the 8 cores; each core owns 6272 padded
    node positions arranged into 49 blocks of 128 (dst = SBUF partition).
  - Per layer every core redundantly computes h_ext = x @ W_ext for ALL nodes
    (x replicated by the host => no exchange before layer 1) and writes 512B
    bf16 rows [h bf16(128) | pad | a_src f32 | pad] to local DRAM.
  - Per block, source features are fetched with dma_gather (int16 indices,
    two 25088-row windows) into [128 dst, t slots, 256] tiles. Slot padding
    points at a dedicated pad row whose a_src is a large negative value, so
    exp() masks padding with no on-chip mask tensor.
  - Softmax runs per-partition on ACT/DVE; the weighted message sum runs on
    the TensorEngine as a chain of diag(alpha) matmuls accumulating in PSUM,
    producing the output TRANSPOSED ([feature, dst]) which feeds layer 2's
    lhsT directly and is transposed back on the host at the end.
  - Between layers, one AllGather (2 chunks) exchanges x2^T bf16 shards.
"""

import hashlib
from contextlib import ExitStack

import ml_dtypes
import numpy as np

# ---------------------------------------------------------------------------
# Configuration
# ---------------------------------------------------------------------------

F = 128              # feature dim (all of F_in, H, F_out)
CORES = 8
ROW = 256            # bf16 elements per h_ext row (512 B)
ACOL = 65            # f32 column (within bitcast row) holding a_src
NEG_SLOPE = 0.2
PAD_S = 2000.0       # pad rows get a_src=-PAD_S, pad dsts a_dst=+PAD_S
GH = 3               # node tiles per h-phase group (PSUM bank holds 3x130 f32)
NQUEUES = 4
GCHUNK = 8           # gathered slot-columns (128 idxs each) per dma_gather


class Cfg:
    def __init__(self, n_nodes, per_core_blocks):
        self.N = n_nodes
        self.NB = per_core_blocks              # blocks of 128 dsts per core
        self.PERP = per_core_blocks * 128      # padded positions per core
        self.PER = n_nodes // CORES            # real nodes per core
        assert self.PER * CORES == n_nodes
        assert self.PERP >= self.PER
        self.NPAD = self.PERP * CORES
        self.WIN = self.PERP * (CORES // 2)    # gather window boundary
        assert self.WIN - 1 < 32768, "window must fit int16"
        self.NT = self.NPAD // 128             # global node tiles
        self.HB = per_core_blocks // 2         # blocks in AllGather chunk A


FULL_CFG = Cfg(50000, 49)


# ---------------------------------------------------------------------------
# Host-side topology preprocessing (pure graph structure, no feature math)
# ---------------------------------------------------------------------------

def build_topology(cfg, edge_index):
    """Returns the shared block structure + per-core gather metadata."""
    src = np.asarray(edge_index[0], dtype=np.int64)
    dst = np.asarray(edge_index[1], dtype=np.int64)
    N, PER, PERP, NB = cfg.N, cfg.PER, cfg.PERP, cfg.NB
    half = N // 2  # orig-id window boundary (cores 0-3 vs 4-7)

    # pad slots gather this position: a pad node (a_src=-PAD_S) in each window
    PAD_A = PER                       # core 0's first pad position, window A
    PAD_B = PERP * (CORES // 2) + PER - cfg.WIN   # core 4's, window B

    # per-core dst permutations and per-block structure
    per_core = []
    orig2pos = np.full(N, -1, dtype=np.int64)
    for k in range(CORES):
        lo = PER * k
        m = (dst >= lo) & (dst < lo + PER)
        s_k = src[m]
        d_k = dst[m] - lo
        w0 = np.bincount(d_k[s_k < half], minlength=PER)
        w1 = np.bincount(d_k[s_k >= half], minlength=PER)
        order = np.lexsort((w0, np.maximum(w0, w1)))
        w0o = np.concatenate([w0[order], np.zeros(PERP - PER, np.int64)])
        w1o = np.concatenate([w1[order], np.zeros(PERP - PER, np.int64)])
        orig2pos[order + lo] = np.arange(PER) + PERP * k
        per_core.append({
            "edges_src": s_k, "edges_dst_local": d_k,
            "S0k": w0o.reshape(NB, 128).max(1),
            "S1k": w1o.reshape(NB, 128).max(1),
        })

    S0 = np.max([c["S0k"] for c in per_core], axis=0).astype(np.int64)
    S1 = np.max([c["S1k"] for c in per_core], axis=0).astype(np.int64)
    T = 1 + S0 + S1                           # self col + both windows

    pos2orig = np.full(cfg.NPAD, -1, dtype=np.int64)
    valid = orig2pos >= 0
    pos2orig[orig2pos[valid]] = np.nonzero(valid)[0]

    # per-core idx arrays
    IA = int(8 * S0.sum())
    IB = int(8 * S1.sum())
    idxA = np.zeros((CORES, 128, max(IA, 16)), np.int16)
    idxB = np.zeros((CORES, 128, max(IB, 16)), np.int16)

    for k in range(CORES):
        c = per_core[k]
        # bucket edges by local dst position
        pos_of_dst = orig2pos[c["edges_dst_local"] + PER * k] - PERP * k
        spos = orig2pos[c["edges_src"]]
        isw0 = spos < cfg.WIN
        bucket0 = [[] for _ in range(PERP)]
        bucket1 = [[] for _ in range(PERP)]
        for p, sp, w in zip(pos_of_dst, spos, isw0):
            (bucket0 if w else bucket1)[p].append(sp)
        aoff = boff = 0
        for b in range(NB):
            s0, s1 = int(S0[b]), int(S1[b])
            flatA = np.full(128 * s0, PAD_A, np.int64)
            flatB = np.full(128 * s1, PAD_B, np.int64)
            for p in range(128):
                g = 128 * b + p
                for s_i, sp in enumerate(bucket0[g]):
                    flatA[s_i * 128 + p] = sp
                for s_i, sp in enumerate(bucket1[g]):
                    flatB[s_i * 128 + p] = sp - cfg.WIN
            # wrap indices into [128, n/16] int16 (16-row layout, replicated)
            for nfl, arr, tgt, off in ((s0, flatA, idxA, aoff),
                                       (s1, flatB, idxB, boff)):
                if nfl == 0:
                    continue
                cols = 8 * nfl
                wrapped = arr.reshape(cols, 16).T.astype(np.int16)
                tgt[k, :, off:off + cols] = np.tile(wrapped, (8, 1))
            aoff += 8 * s0
            boff += 8 * s1

    stats = {
        "real_edges": int(sum(len(c["edges_src"]) for c in per_core)) + N,
        "padded_edges": int((T.sum()) * 128 * CORES),
    }
    return {
        "S0": S0, "S1": S1, "T": T, "IA": IA, "IB": IB,
        "idxA": idxA, "idxB": idxB,
        "orig2pos": orig2pos, "pos2orig": pos2orig, "stats": stats,
    }


# ---------------------------------------------------------------------------
# Bass program
# ---------------------------------------------------------------------------

def build_program(cfg, topo, queue_map=None):
    """queue_map: construction-index -> queue_num. Tile assigns the SWDGE
    DMASW sem lanes in SCHEDULED order; a sem lane is locked to the queue of
    the first gather that bumps it, so queue_num must follow the scheduled
    rotation. We discover it with a first compile pass (see _get_program)."""
    import concourse.bacc as bacc
    import concourse.mybir as mybir
    import concourse.tile as tile

    dt = mybir.dt
    S0, S1, T = topo["S0"], topo["S1"], topo["T"]
    IA, IB = topo["IA"], topo["IB"]
    NPAD, PERP, WIN, NB, NT = cfg.NPAD, cfg.PERP, cfg.WIN, cfg.NB, cfg.NT
    PER = cfg.PER
    # h-row exchange chunks (in blocks): each chunk's AllGather fires as
    # soon as its h_own rows are written; the tail chunks are small so only
    # a tiny AllGather remains after the layer-1 edge loop finishes (the
    # final one carries just the pad block)
    nb1 = NB - 1
    t1 = max(1, (9 * nb1) // 20)
    CH = [(0, t1), (t1, 2 * t1), (2 * t1, nb1), (nb1, NB)]
    CH = [(a, b) for a, b in CH if b > a]

    nc = bacc.Bacc("TRN2", target_bir_lowering=False, debug=False,
                   enable_asserts=False, num_devices=CORES,
                   num_swdge_queues=NQUEUES,
                   dynamic_dma_scratch_size=32768)

    # --- kernel I/O ---
    xTg = nc.dram_tensor("xTg", [F, NPAD], dt.bfloat16, kind="ExternalInput")
    xTo = nc.dram_tensor("xTo", [F, PERP], dt.bfloat16, kind="ExternalInput")
    W1e = nc.dram_tensor("W1e", [F, 130], dt.bfloat16, kind="ExternalInput")
    W2e = nc.dram_tensor("W2e", [F, 130], dt.bfloat16, kind="ExternalInput")
    idxA_d = nc.dram_tensor("idxA", [128, max(IA, 16)], dt.int16, kind="ExternalInput")
    idxB_d = nc.dram_tensor("idxB", [128, max(IB, 16)], dt.int16, kind="ExternalInput")
    b1c_d = nc.dram_tensor("b1c", [128, 1], dt.float32, kind="ExternalInput")
    b2c_d = nc.dram_tensor("b2c", [128, 1], dt.float32, kind="ExternalInput")
    eyeb_d = nc.dram_tensor("eyeb", [128, 128], dt.bfloat16, kind="ExternalInput")
    x2pad_d = nc.dram_tensor("x2pad", [128, 1], dt.bfloat16, kind="ExternalInput")
    outT_d = nc.dram_tensor("outT", [F, PERP], dt.float32, kind="ExternalOutput")

    # --- internal DRAM ---
    h1x = nc.dram_tensor("h1x", [NPAD, ROW], dt.bfloat16)
    h1o = nc.dram_tensor("h1o", [PERP, ROW], dt.bfloat16)
    h2x = nc.dram_tensor("h2x", [NPAD, ROW], dt.bfloat16)
    h2o = nc.dram_tensor("h2o", [PERP, ROW], dt.bfloat16)
    # packed (132-col) h2 rows: shard staging + AllGather outputs
    cc2_in = nc.dram_tensor("cc2_in", [PERP, 132], dt.bfloat16)
    cc2_outs = [nc.dram_tensor(f"cc2_{i}out",
                               [CORES, 128 * (b - a) * 132], dt.bfloat16,
                               addr_space="Shared")
                for i, (a, b) in enumerate(CH)]

    with tile.TileContext(nc) as tc, ExitStack() as ctx:
        P = ctx.enter_context(tc.tile_pool(name="persist", bufs=1))
        hp = ctx.enter_context(tc.tile_pool(name="hp", bufs=4))
        php = ctx.enter_context(tc.tile_pool(name="php", bufs=2, space="PSUM"))
        php = ctx.enter_context(tc.tile_pool(name="php", bufs=2, space="PSUM"))
        pop = ctx.enter_context(tc.tile_pool(name="pop", bufs=2, space="PSUM"))
        pep = ctx.enter_context(tc.tile_pool(name="pep", bufs=3, space="PSUM"))
        gp = ctx.enter_context(tc.tile_pool(name="gp", bufs=4))
        dp = ctx.enter_context(tc.tile_pool(name="dp", bufs=3))
        sp = ctx.enter_context(tc.tile_pool(name="sp", bufs=6))
        op = ctx.enter_context(tc.tile_pool(name="op", bufs=3))

        # persistent SBUF
        idxA_s = P.tile([128, max(IA, 16)], dt.int16)
        idxB_s = P.tile([128, max(IB, 16)], dt.int16)
        W1e_s = P.tile([F, 130], dt.bfloat16)
        W2e_s = P.tile([F, 130], dt.bfloat16)
        b1c_s = P.tile([128, 1], dt.float32)
        b2c_s = P.tile([128, 1], dt.float32)
        eyeb_s = P.tile([128, 128], dt.bfloat16)
        x2pad_s = P.tile([128, 1], dt.bfloat16)
        adst1 = P.tile([128, NB], dt.float32)
        adst2 = P.tile([128, NB], dt.float32)
        x2t = [P.tile([F, 128 * (b - a)], dt.bfloat16, name=f"x2c{i}")
               for i, (a, b) in enumerate(CH)]

        def x2ap(b):
            for (a, bb), tile_ in zip(CH, x2t):
                if a <= b < bb:
                    return tile_[:, 128 * (b - a):128 * (b - a + 1)]
            raise AssertionError(b)

        nc.sync.dma_start(idxA_s[:], idxA_d[:])
        nc.sync.dma_start(idxB_s[:], idxB_d[:])
        nc.sync.dma_start(W1e_s[:], W1e[:])
        nc.sync.dma_start(W2e_s[:], W2e[:])
        nc.sync.dma_start(b1c_s[:], b1c_d[:])
        nc.sync.dma_start(b2c_s[:], b2c_d[:])
        nc.sync.dma_start(eyeb_s[:], eyeb_d[:])
        nc.sync.dma_start(x2pad_s[:], x2pad_d[:])

        def h_group(xt_ap, hx_dram, row0, gn, We_s, use_act):
            """h_ext for gn node tiles (lhsT columns of xt_ap) -> 512B rows."""
            ps = php.tile([128, GH, 130], dt.float32, tag="ps")
            for j in range(gn):
                nc.tensor.matmul(ps[:, j, :],
                                 xt_ap[:, 128 * j:128 * (j + 1)], We_s[:])
            hxt = hp.tile([128, GH, ROW], dt.bfloat16, tag="hx")
            if use_act:
                nc.scalar.activation(hxt[:, 0:gn, 0:130], ps[:, 0:gn, 0:130],
                                     mybir.ActivationFunctionType.Copy)
            else:
                nc.vector.tensor_copy(hxt[:, 0:gn, 0:130], ps[:, 0:gn, 0:130])
            h32 = hxt[:].bitcast(dt.float32)    # [128, GH, 128]
            nc.vector.tensor_copy(h32[:, 0:gn, ACOL:ACOL + 1],
                                  ps[:, 0:gn, 128:129])
            nc.sync.dma_start(
                hx_dram[row0:row0 + 128 * gn, 0:132].rearrange(
                    "(g p) c -> p g c", p=128), hxt[:, 0:gn, 0:132])

        def h_global_l1():
            for i, t0 in enumerate(range(0, NT, GH)):
                gn = min(GH, NT - t0)
                xt = hp.tile([F, 128 * GH], dt.bfloat16, tag="xt")
                nc.sync.dma_start(xt[:, 0:128 * gn],
                                  xTg[:, 128 * t0:128 * (t0 + gn)])
                h_group(xt[:], h1x, 128 * t0, gn, W1e_s, use_act=(i % 2 == 0))

        def h_own_block(xsrc_kind, b, ho_dram, cc_in, We_s, adst_s):
            if xsrc_kind == "xTo":
                xt = hp.tile([F, 128], dt.bfloat16, tag="xto")
                nc.sync.dma_start(xt[:], xTo[:, 128 * b:128 * (b + 1)])
                lhs = xt[:]
            else:
                lhs = x2ap(b)
            ps = pop.tile([128, 130], dt.float32, tag="pso")
            nc.tensor.matmul(ps[:], lhs, We_s[:])
            hxt = hp.tile([128, ROW], dt.bfloat16, tag="hxo")
            nc.scalar.activation(hxt[:, 0:130], ps[:, 0:130],
                                 mybir.ActivationFunctionType.Copy)
            h32 = hxt[:].bitcast(dt.float32)
            nc.vector.tensor_copy(h32[:, ACOL:ACOL + 1], ps[:, 128:129])
            nc.vector.tensor_copy(adst_s[:, b:b + 1], ps[:, 129:130])
            nc.scalar.dma_start(ho_dram[128 * b:128 * (b + 1), 0:132],
                                hxt[:, 0:132])
            if cc_in is not None:
                nc.scalar.dma_start(cc_in[128 * b:128 * (b + 1), :],
                                    hxt[:, 0:132])

        def exchange_chunk(i, cc_in, cc_outs_l, hx_dram):
            """AllGather one packed h-row chunk, then DMA-repack every
            rank's rows into the 512B-stride gather layout (window-A ranks
            first)."""
            (b0, b1), cc_out_t = CH[i], cc_outs_l[i]
            rows = 128 * (b1 - b0)
            nc.gpsimd.collective_compute(
                "AllGather", mybir.AluOpType.bypass,
                replica_groups=[list(range(CORES))],
                ins=[cc_in[128 * b0:128 * b1, :].opt()],
                outs=[cc_out_t[:].opt()])
            for r in list(range(CORES // 2)) + list(range(CORES // 2, CORES)):
                nc.sync.dma_start(
                    hx_dram[r * PERP + 128 * b0:
                            r * PERP + 128 * b0 + rows, 0:132],
                    cc_out_t[r, :].rearrange("(n c) -> n c", c=132))

        def exchange(cc_in, cc_outs_l, hx_dram):
            for i in range(len(CH)):
                exchange_chunk(i, cc_in, cc_outs_l, hx_dram)

        gq = [0]
        gather_insts = []

        def qnum():
            i = gq[0]
            gq[0] += 1
            return queue_map.get(i, 0) if queue_map else 0

        def edge_phase(hx_dram, ho_dram, adst_s, bc_s, layer):
            aoff = boff = 0
            for b in range(NB):
                s0, s1, t = int(S0[b]), int(S1[b]), int(T[b])
                G = gp.tile([128, t, ROW], dt.bfloat16, tag="G")
                nc.sync.dma_start(G[:, 0, 0:132],
                                  ho_dram[128 * b:128 * (b + 1), 0:132])
                for c0 in range(0, s0, GCHUNK):
                    cn = min(GCHUNK, s0 - c0)
                    gather_insts.append(nc.gpsimd.dma_gather(
                        G[:, 1 + c0:1 + c0 + cn, :], hx_dram[0:WIN, :],
                        idxA_s[:, aoff + 8 * c0:aoff + 8 * (c0 + cn)],
                        128 * cn, 128 * cn, ROW, queue_num=qnum()))
                for c0 in range(0, s1, GCHUNK):
                    cn = min(GCHUNK, s1 - c0)
                    gather_insts.append(nc.gpsimd.dma_gather(
                        G[:, 1 + s0 + c0:1 + s0 + c0 + cn, :],
                        hx_dram[WIN:NPAD, :],
                        idxB_s[:, boff + 8 * c0:boff + 8 * (c0 + cn)],
                        128 * cn, 128 * cn, ROW, queue_num=qnum()))
                G32 = G[:].bitcast(dt.float32)      # [128, t, 128]
                # e = a_src + a_dst ; leaky ; exp (padding: a_src=-PAD_S -> 0)
                # NB: DVE's AP-scalar operand path costs ~4.3us flat, so feed
                # a_dst as a stride-0 broadcast tensor operand instead.
                E = sp.tile([128, t], dt.float32, tag="E")
                nc.vector.tensor_tensor(
                    E[:], G32[:, :, ACOL],
                    adst_s[:, b:b + 1].broadcast_to([128, t]),
                    mybir.AluOpType.add)
                EL = sp.tile([128, t], dt.float32, tag="EL")
                nc.vector.scalar_tensor_tensor(
                    EL[:], E[:], NEG_SLOPE, E[:],
                    mybir.AluOpType.mult, mybir.AluOpType.max)
                EX = sp.tile([128, t], dt.float32, tag="EX")
                den = sp.tile([128, 1], dt.float32, tag="den")
                nc.scalar.activation(EX[:], EL[:],
                                     mybir.ActivationFunctionType.Exp,
                                     accum_out=den[:])
                rec = sp.tile([128, 1], dt.float32, tag="rec")
                nc.vector.reciprocal(rec[:], den[:])
                EXnb = sp.tile([128, t], dt.bfloat16, tag="EXnb")
                nc.scalar.activation(EXnb[:], EX[:],
                                     mybir.ActivationFunctionType.Copy,
                                     scale=rec[:, 0:1])
                # D[:, s, :] = diag-stack of normalized alphas
                D = dp.tile([128, t, F], dt.bfloat16, tag="D")
                nc.vector.tensor_tensor(
                    D[:], EXnb[:].unsqueeze(2).broadcast_to([128, t, F]),
                    eyeb_s[:].unsqueeze(1).broadcast_to([128, t, F]),
                    mybir.AluOpType.mult)
                # psum[f, d] = sum_s G_s^T @ D_s  (transposed GAT output)
                ps = pep.tile([128, 128], dt.float32, tag="ps")
                for s in range(t):
                    nc.tensor.matmul(ps[:], G[:, s, 0:F], D[:, s, :],
                                     start=(s == 0), stop=(s == t - 1))
                if layer == 1:
                    nc.scalar.activation(x2ap(b), ps[:],
                                         mybir.ActivationFunctionType.Relu,
                                         bias=bc_s[:, 0:1])
                    if b < NB - 1:
                        h_own_block("x2", b, h2o, cc2_in, W2e_s, adst2)
                    for i in range(len(CH) - 1):
                        if b == min(CH[i][1] + 2, NB - 2):
                            exchange_chunk(i, cc2_in, cc2_outs, h2x)
                else:
                    ot = op.tile([128, 128], dt.float32, tag="ot")
                    nc.scalar.activation(ot[:], ps[:],
                                         mybir.ActivationFunctionType.Relu,
                                         bias=bc_s[:, 0:1])
                    nc.sync.dma_start(outT_d[:, 128 * b:128 * (b + 1)], ot[:])
                aoff += 8 * s0
                boff += 8 * s1

        # ---- layer 1: replicated local h (engines are idle at start) ----
        h_global_l1()
        for b in range(NB):
            h_own_block("xTo", b, h1o, None, W1e_s, adst1)
        edge_phase(h1x, h1o, adst1, b1c_s, layer=1)
        # (layer-2 h_own for blocks 0..NB-2 is interleaved into edge_phase)

        # pad columns of x2 get the crafted pad vector (a_src2=-S, a_dst2=+S);
        # then the last block's h_own runs over the patched columns
        lastw = 128 * (CH[-1][1] - CH[-1][0])
        nc.vector.tensor_copy(
            x2t[-1][:, lastw - (PERP - PER):lastw],
            x2pad_s[:].broadcast_to([128, PERP - PER]))
        h_own_block("x2", NB - 1, h2o, cc2_in, W2e_s, adst2)

        # ---- layer 2 (chunks 0..n-2 already exchanged mid-edge-phase) ----
        exchange_chunk(len(CH) - 1, cc2_in, cc2_outs, h2x)
        edge_phase(h2x, h2o, adst2, b2c_s, layer=2)

    nc.compile()
    # final (scheduled) order of the gather instructions, by construction idx
    name2ci = {gi.ins.name: ci for ci, gi in enumerate(gather_insts)}
    sched = []

    def _walk(bb):
        for inst in bb.instructions:
            ci = name2ci.get(inst.name)
            if ci is not None:
                sched.append(ci)
            body = getattr(inst, "body_bb", None)
            if body is not None:
                _walk(body)

    for bb in nc.m.functions[0].blocks:
        _walk(bb)
    sched_q = {ci: pos % NQUEUES for pos, ci in enumerate(sched)}
    return nc, sched_q


# ---------------------------------------------------------------------------
# Host orchestration
# ---------------------------------------------------------------------------

def make_inputs(cfg, topo, x, W1, as1, ad1, b1, W2, as2, ad2, b2):
    N, NPAD, PERP, PER = cfg.N, cfg.NPAD, cfg.PERP, cfg.PER
    bf16 = ml_dtypes.bfloat16
    pos2orig = topo["pos2orig"]

    def wext(W, a_s, a_d):
        W = np.asarray(W, np.float64)
        return np.concatenate(
            [W, (W @ np.asarray(a_s, np.float64))[:, None],
             (W @ np.asarray(a_d, np.float64))[:, None]], axis=1)

    W1f = wext(W1, as1, ad1)            # [F, 130] f64
    W2f = wext(W2, as2, ad2)

    def pad_vec(Wf):
        # least-norm x with x@Wa_s = -PAD_S and x@Wa_d = +PAD_S
        A = Wf[:, 128:130].T            # [2, F]
        rhs = np.array([-PAD_S, PAD_S], np.float64)
        return (A.T @ np.linalg.solve(A @ A.T, rhs))

    x1p = pad_vec(W1f)
    x2p = pad_vec(W2f)

    xT = np.empty((F, NPAD), bf16)
    xT[:] = x1p[:, None].astype(bf16)   # pad columns get the mask vector
    valid = pos2orig >= 0
    xT[:, valid] = np.asarray(x, np.float32)[pos2orig[valid]].T.astype(bf16)

    in_maps = []
    for k in range(CORES):
        in_maps.append({
            "xTg": xT,
            "xTo": np.ascontiguousarray(xT[:, PERP * k:PERP * (k + 1)]),
            "W1e": W1f.astype(bf16), "W2e": W2f.astype(bf16),
            "idxA": topo["idxA"][k],
            "idxB": topo["idxB"][k],
            "b1c": np.asarray(b1, np.float32)[:, None],
            "b2c": np.asarray(b2, np.float32)[:, None],
            "eyeb": np.eye(128, dtype=bf16),
            "x2pad": x2p[:, None].astype(bf16),
        })
    return in_maps


_CACHE = {}


def _get_program(cfg, edge_index):
    key = hashlib.sha1(np.ascontiguousarray(edge_index).tobytes()).hexdigest()
    if key not in _CACHE:
        topo = build_topology(cfg, edge_index)
        # pass 1 discovers the scheduler's gather order (=> DMASW sem lanes);
        # pass 2 rebuilds with queue_num matching it. Iterate to convergence
        # in case queue choice perturbs the schedule.
        qmap = None
        for _ in range(4):
            nc, sched_q = build_program(cfg, topo, qmap)
            if qmap == sched_q:
                break
            qmap = sched_q
        _CACHE[key] = (topo, nc)
    return _CACHE[key]


def run(cfg, inputs, trace=False):
    from concourse.bass_utils import run_bass_kernel_spmd

    topo, nc = _get_program(cfg, inputs["edge_index"])
    in_maps = make_inputs(
        cfg, topo, inputs["x"],
        inputs["W1"], inputs["att_src1"], inputs["att_dst1"], inputs["b1"],
        inputs["W2"], inputs["att_src2"], inputs["att_dst2"], inputs["b2"])
    res = run_bass_kernel_spmd(nc, in_maps, list(range(CORES)), trace=trace)

    full = np.zeros((cfg.N, F), np.float32)
    pos2orig = topo["pos2orig"]
    for k in range(CORES):
        o = np.asarray(res.results[k]["outT"], np.float32)   # [F, PERP]
        po = pos2orig[cfg.PERP * k:cfg.PERP * (k + 1)]
        m = po >= 0
        full[po[m]] = o[:, m].T
    return full, res


def kernel(**inputs) -> np.ndarray:
    out, _ = run(FULL_CFG, inputs)
    return out
